# revision 24
# baseline (speedup 1.0000x reference)
"""Brenier-map ICNN gradient kernel for Trainium2 (8 NeuronCores, data parallel).

Computes grad_u of sum(ICNN(u)) for the 5-layer input-convex network in the
reference: forward MLP with exp() weights + hand-derived backward pass.

Design:
  - Pure batch data-parallelism: each core gets 8192 of 65536 samples.
  - Host precomputes exp(weights), transposes, and bf16 casts.
  - On-chip layout keeps hidden units on partitions and samples on the free
    dim ("transposed" activations), so the z-chain (forward and backward)
    needs no transposes at all.  The gradient accumulation runs with the
    backward deltas as the *stationary* matmul operand, which produces the
    output in natural [samples, 64] layout directly.
  - All matmuls bf16 with fp32 PSUM accumulation.
  - LeakyReLU+bias is a single ACT-engine Prelu per tile (alpha=0.2); the
    derivative mask m = max(psum > -b, 0.2) is a single fused DVE
    tensor_scalar; backward applies it with one tensor_tensor per tile.
    Layer 0's combined factor a0*lrelu'(s0) is just Prelu(a0); its extra
    factor 2 is folded into the gradient-side copy of exp(wu0).
"""

import numpy as np
from contextlib import ExitStack

import concourse.bacc as bacc
import concourse.mybir as mybir
import concourse.tile as tile
from concourse.bass import ds
from concourse.bass_utils import run_bass_kernel_spmd
from ml_dtypes import bfloat16

B, D, H = 65536, 64, 512
N_CORES = 8
B_CORE = B // N_CORES        # 8192 samples per core
CHUNK = 512                  # samples per pipeline chunk
N_CHUNKS = B_CORE // CHUNK   # 16
NT = H // 128                # 4 hidden-dim tiles of 128
ALPHA = 0.2

F32 = mybir.dt.float32
BF16 = mybir.dt.bfloat16
AF = mybir.ActivationFunctionType
OP = mybir.AluOpType

_PROGRAMS = {}


def _body(ctx, tc, uT_d, euT_d, eu4T_d, ezT_d, ezn_d, eu4_d, eun_d,
          bias_d, negb_d, negb4_d, out_d):
    nc = tc.nc
    wpool = ctx.enter_context(tc.tile_pool(name="weights", bufs=1))
    acts = ctx.enter_context(tc.tile_pool(name="acts", bufs=2))
    dspool = ctx.enter_context(tc.tile_pool(name="dsp", bufs=3))
    iop = ctx.enter_context(tc.tile_pool(name="io", bufs=2))
    utp = ctx.enter_context(tc.tile_pool(name="utp", bufs=3))
    pps = ctx.enter_context(tc.tile_pool(name="pps", bufs=4, space="PSUM"))
    pps4 = ctx.enter_context(tc.tile_pool(name="pps4", bufs=1, space="PSUM"))
    pdz = ctx.enter_context(tc.tile_pool(name="pdz", bufs=2, space="PSUM"))
    pgu = ctx.enter_context(tc.tile_pool(name="pgu", bufs=1, space="PSUM"))

    # ---- resident inputs (loaded once; uT streams per chunk) ----
    # Small tensors first so chunk-0 isn't gated behind the 6MB of wz
    # weights; wz loads are split per layer in first-use order.
    bias_s = wpool.tile([128, 4, NT], F32)
    nc.sync.dma_start(out=bias_s, in_=bias_d.rearrange("i (j p) -> p i j", p=128))
    negb_s = wpool.tile([128, 4, NT], F32)
    nc.sync.dma_start(out=negb_s, in_=negb_d.rearrange("i (j p) -> p i j", p=128))
    negb4_s = wpool.tile([1, 1], F32)
    nc.sync.dma_start(out=negb4_s, in_=negb4_d)
    euP_s = wpool.tile([128, 8 * 128], BF16)
    nc.sync.dma_start(out=euP_s, in_=euT_d)
    eu4T_s = wpool.tile([D, 1], BF16)
    nc.sync.dma_start(out=eu4T_s, in_=eu4T_d)
    ones_s = wpool.tile([128, 1], BF16)
    nc.vector.memset(ones_s, 1.0)
    eu4_s = wpool.tile([1, D], BF16)
    nc.sync.dma_start(out=eu4_s, in_=eu4_d)
    eun_s = wpool.tile([128, 4 * NT, D], BF16)
    nc.gpsimd.dma_start(out=eun_s, in_=eun_d.rearrange("b p d -> p b d"))
    zeros_s = wpool.tile([1, NT * D], BF16)
    nc.vector.memset(zeros_s, 0.0)
    ezT_v = ezT_d.rearrange("i (k p) n -> i p k n", p=128)
    ezT_s = wpool.tile([128, 3, NT, H], BF16)
    for i in range(3):
        nc.sync.dma_start(out=ezT_s[:, i], in_=ezT_v[i])
    ezn_v = ezn_d.rearrange("i (k p) n -> i p k n", p=128)
    ezn_s = wpool.tile([128, 3, NT, H], BF16)
    for i in (2, 1, 0):
        nc.gpsimd.dma_start(out=ezn_s[:, i], in_=ezn_v[i])

    out_v = out_d.rearrange("(c g p) d -> c p g d", g=NT, p=128)

    for c in range(N_CHUNKS):
        cs = ds(c * CHUNK, CHUNK)
        ut = utp.tile([128, CHUNK], BF16, name="ut")
        nc.gpsimd.dma_start(out=ut, in_=uT_d[:, cs])

        # ---------------- forward ----------------
        # u-path matmuls run as row-group pairs: lhsT halves live on SBUF
        # partitions 0-63 / 64-127 (euP), rhs is uT duplicated on both
        # halves, tile_position (0,0)/(64,0) -> the two K=64 matmuls
        # occupy disjoint quadrant rows and overlap on the PE array.
        # layer 0: z0 = lrelu(u @ E0.T + b0)^2; g0 = a0 * lrelu'(s0)
        z0 = acts.tile([128, NT, CHUNK], BF16, name="z0")
        g0 = acts.tile([128, NT, CHUNK], BF16, name="g0")
        for jp in range(NT // 2):
            pcols = ds((0 * 2 + jp) * 128, 128)
            sps = [pps.tile([128, CHUNK], F32, name="sp") for _ in range(2)]
            nc.tensor.matmul(sps[0], euP_s[0:64, pcols], ut[0:64, :],
                             tile_position=(0, 0), start=True, stop=True)
            nc.tensor.matmul(sps[1], euP_s[64:128, pcols], ut[64:128, :],
                             tile_position=(64, 0), start=True, stop=True)
            for h, sp in enumerate(sps):
                j = 2 * jp + h
                a0 = acts.tile([128, CHUNK], BF16, name="a0")
                nc.scalar.activation(a0, sp, AF.Prelu,
                                     bias=bias_s[:, 0, j:j + 1], alpha=ALPHA)
                nc.scalar.square(z0[:, j, :], a0)
                nc.scalar.activation(g0[:, j, :], a0, AF.Prelu, alpha=ALPHA)

        # layers 1..3: z_i = lrelu(u @ Eu_i.T + z_{i-1} @ Ez_i.T + b_i)
        zp = z0
        ms = {}
        for i in (1, 2, 3):
            zi = acts.tile([128, NT, CHUNK], BF16, name=f"z{i}")
            mi = acts.tile([128, NT, CHUNK], BF16, name=f"m{i}")
            for jp in range(NT // 2):
                pcols = ds((i * 2 + jp) * 128, 128)
                sps = [pps.tile([128, CHUNK], F32, name="sp") for _ in range(2)]
                nc.tensor.matmul(sps[0], euP_s[0:64, pcols], ut[0:64, :],
                                 tile_position=(0, 0), start=True, stop=False)
                nc.tensor.matmul(sps[1], euP_s[64:128, pcols], ut[64:128, :],
                                 tile_position=(64, 0), start=True, stop=False)
                for h, sp in enumerate(sps):
                    j = 2 * jp + h
                    for k in range(NT):
                        nc.tensor.matmul(sp, ezT_s[:, i - 1, k, ds(j * 128, 128)],
                                         zp[:, k, :], start=False,
                                         stop=(k == NT - 1))
                    nc.vector.tensor_scalar(mi[:, j, :], sp,
                                            negb_s[:, i, j:j + 1],
                                            ALPHA, OP.is_gt, OP.max)
                    nc.scalar.activation(zi[:, j, :], sp, AF.Prelu,
                                         bias=bias_s[:, i, j:j + 1], alpha=ALPHA)
            zp = zi
            ms[i] = mi

        # layer 4 (scalar head): only the lrelu' mask ds4 is needed
        s4p = pps4.tile([1, CHUNK], F32, name="s4p")
        nc.tensor.matmul(s4p, eu4T_s, ut[0:64, :], start=True, stop=False)
        for k in range(NT):
            nc.tensor.matmul(s4p, ones_s, zp[:, k, :],
                             start=False, stop=(k == NT - 1))
        ds4 = dspool.tile([1, CHUNK], BF16, name="ds4")
        nc.vector.tensor_scalar(ds4, s4p, negb4_s, ALPHA, OP.is_gt, OP.max)

        # ---------------- backward ----------------
        # grad accumulator in natural [samples, 64] layout; backward deltas
        # are the stationary operand so no output transpose is needed.
        gup = pgu.tile([128, NT, D], F32, name="gup")
        # single accumulation group over the whole bank: zero it with one
        # K=1 matmul (start=True), then everything accumulates into it.
        nc.tensor.matmul(gup[:, :, :], zeros_s[:, 0:128], zeros_s,
                         start=True, stop=False)
        for g in range(NT):
            nc.tensor.matmul(gup[:, g, :], ds4[:, ds(g * 128, 128)], eu4_s,
                             start=False, stop=False)

        # ds3 = broadcast(ds4) * m3   (Ez4 folded into layer-3 weights)
        bds4 = dspool.tile([128, CHUNK], BF16, name="bds4")
        nc.gpsimd.partition_broadcast(bds4, ds4)
        dst = {}
        for j in range(NT):
            dd = dspool.tile([128, CHUNK], BF16, name=f"ds3_{j}")
            nc.vector.tensor_tensor(dd, bds4, ms[3][:, j, :], OP.mult)
            dst[j] = dd

        for i in (3, 2, 1):
            # gu += ds_i @ Eu_i
            for j in range(NT):
                for g in range(NT):
                    nc.tensor.matmul(gup[:, g, :], dst[j][:, ds(g * 128, 128)],
                                     eun_s[:, i * NT + j, :],
                                     start=False, stop=False)
            # dz_{i-1} = ds_i @ Ez_i ; ds_{i-1} = dz * m_{i-1} (g0 for i==1)
            nxt = {}
            for j in range(NT):
                dzp = pdz.tile([128, CHUNK], F32, name="dzp")
                for k in range(NT):
                    nc.tensor.matmul(dzp, ezn_s[:, i - 1, k, ds(j * 128, 128)],
                                     dst[k], start=(k == 0), stop=(k == NT - 1))
                dd = dspool.tile([128, CHUNK], BF16, name=f"ds_{j}")
                mul = g0[:, j, :] if i == 1 else ms[i - 1][:, j, :]
                nc.vector.tensor_tensor(dd, dzp, mul, OP.mult)
                nxt[j] = dd
            dst = nxt

        # gu += ds0 @ (2*E0)  (factor 2 folded into eun block 0 on the host)
        for j in range(NT):
            for g in range(NT):
                nc.tensor.matmul(gup[:, g, :], dst[j][:, ds(g * 128, 128)],
                                 eun_s[:, j, :], start=False,
                                 stop=(j == NT - 1 and g == NT - 1))

        gsb = iop.tile([128, NT, D], F32, name="gsb")
        nc.scalar.copy(gsb, gup)
        nc.sync.dma_start(out=out_v[c], in_=gsb)


def _build_program():
    nc = bacc.Bacc("TRN2", target_bir_lowering=False, debug=False,
                   enable_asserts=False)
    uT_d = nc.dram_tensor("uT", [128, B_CORE], BF16, kind="ExternalInput").ap()
    euT_d = nc.dram_tensor("euT", [128, 8 * 128], BF16, kind="ExternalInput").ap()
    eu4T_d = nc.dram_tensor("eu4T", [D, 1], BF16, kind="ExternalInput").ap()
    ezT_d = nc.dram_tensor("ezT", [3, H, H], BF16, kind="ExternalInput").ap()
    ezn_d = nc.dram_tensor("ezn", [3, H, H], BF16, kind="ExternalInput").ap()
    eu4_d = nc.dram_tensor("eu4", [1, D], BF16, kind="ExternalInput").ap()
    eun_d = nc.dram_tensor("eun", [4 * NT, 128, D], BF16, kind="ExternalInput").ap()
    bias_d = nc.dram_tensor("bias", [4, H], F32, kind="ExternalInput").ap()
    negb_d = nc.dram_tensor("negb", [4, H], F32, kind="ExternalInput").ap()
    negb4_d = nc.dram_tensor("negb4", [1, 1], F32, kind="ExternalInput").ap()
    out_d = nc.dram_tensor("out", [B_CORE, D], F32, kind="ExternalOutput").ap()

    with ExitStack() as ctx:
        tc = ctx.enter_context(tile.TileContext(nc))
        _body(ctx, tc, uT_d, euT_d, eu4T_d, ezT_d, ezn_d, eu4_d, eun_d,
              bias_d, negb_d, negb4_d, out_d)
    nc.compile()
    return nc


def _get_program():
    if "main" not in _PROGRAMS:
        _PROGRAMS["main"] = _build_program()
    return _PROGRAMS["main"]


def _prepare_in_maps(inputs):
    u = np.asarray(inputs["u"], dtype=np.float32)
    wu = [np.asarray(inputs[f"wu{i}"], np.float32) for i in range(5)]
    wz = {i: np.asarray(inputs[f"wz{i}"], np.float32) for i in (1, 2, 3, 4)}
    b = [np.asarray(inputs[f"b{i}"], np.float32) for i in range(5)]

    Eu = [np.exp(w) for w in wu]           # [H, D]; Eu[4] is [1, D]
    Ez = {i: np.exp(wz[i]) for i in wz}    # [H, H]; Ez[4] is [1, H]

    # Fold Ez4 into layer 3 (the lrelu' mask is scale-invariant): layer-3
    # rows are scaled by Ez4, the L4 z-path weight becomes all-ones, and
    # backward's dz3 = broadcast(ds4).
    sc = Ez[4][0]                                                  # [H]
    Eu3s = Eu[3] * sc[:, None]
    Ez3s = Ez[3] * sc[:, None]
    b3s = b[3] * sc
    euT = np.concatenate(
        [Eu[0].T, Eu[1].T, Eu[2].T, Eu3s.T], axis=1)               # [D, 4H]
    # row-group pairs: pair p covers u-path tiles (2p, 2p+1) of the flat
    # (layer, j) order; halves live on partition rows 0-63 / 64-127.
    euP = np.empty((128, 8 * 128), np.float32)
    for p in range(8):
        euP[:D, p * 128:(p + 1) * 128] = euT[:, (2 * p) * 128:(2 * p + 1) * 128]
        euP[D:, p * 128:(p + 1) * 128] = euT[:, (2 * p + 1) * 128:(2 * p + 2) * 128]
    bias = np.stack([b[0], b[1], b[2], b3s])                       # [4, H]

    bf = lambda x: np.ascontiguousarray(x, dtype=np.float32).astype(bfloat16)
    f32 = lambda x: np.ascontiguousarray(x, dtype=np.float32)
    weights = {
        "euT": bf(euP),
        "eu4T": bf(Eu[4].T),
        "ezT": bf(np.stack([Ez[1].T, Ez[2].T, Ez3s.T])),
        "ezn": bf(np.stack([Ez[1], Ez[2], Ez3s])),
        "eu4": bf(Eu[4]),
        "eun": bf(np.concatenate([2.0 * Eu[0], Eu[1], Eu[2], Eu3s],
                                 axis=0).reshape(4 * NT, 128, D)),
        "bias": f32(bias),
        "negb": f32(-bias),
        "negb4": f32(-b[4].reshape(1, 1)),
    }

    in_maps = []
    for core in range(N_CORES):
        ush = u[core * B_CORE:(core + 1) * B_CORE]
        uT2 = np.concatenate([ush.T, ush.T], axis=0)               # [128, Bc]
        in_maps.append({"uT": bf(uT2), **weights})
    return in_maps


def kernel(**inputs):
    in_maps = _prepare_in_maps(inputs)
    nc = _get_program()
    res = run_bass_kernel_spmd(nc, in_maps, core_ids=list(range(N_CORES)))
    return np.concatenate([res.results[i]["out"] for i in range(N_CORES)],
                          axis=0)



# revision 27
# speedup vs baseline: 2778.8816x; 2778.8816x over previous
"""Brenier-map ICNN gradient kernel for Trainium2 (8 NeuronCores, data parallel).

Computes grad_u of sum(ICNN(u)) for the 5-layer input-convex network in the
reference: forward MLP with exp() weights + hand-derived backward pass.

Design:
  - Pure batch data-parallelism: each core gets 8192 of 65536 samples,
    weights replicated; no collectives.
  - Host precomputes exp(weights), transposes, and bf16 casts.
  - On-chip layout keeps hidden units on partitions and samples on the free
    dim ("transposed" activations), so the z-chain (forward and backward)
    needs no transposes at all.  The gradient accumulation runs with the
    backward deltas as the *stationary* matmul operand, which produces the
    output in natural [samples, 64] layout directly.
  - All matmuls bf16 with fp32 PSUM accumulation (absmax-rel err ~5e-3).
  - LeakyReLU+bias is a single ACT-engine Prelu per tile (alpha=0.2); the
    derivative mask m = max(psum > -b, 0.2) is a single fused DVE
    tensor_scalar; backward applies it with one tensor_tensor per tile.
    Layer 0's combined factor a0*lrelu'(s0) is just Prelu(a0); its extra
    factor 2 is folded into the gradient-side copy of exp(wu0).
  - The K=64 u-path matmuls run as row-group pairs (tile_position (0,0) /
    (64,0)) so two half-height matmuls overlap on the PE array.
  - exp(wz4) is folded into layer 3 on the host (the lrelu' mask is
    scale-invariant), so the scalar head's z-weight is all-ones and
    backward's dz3 is just a gpsimd partition_broadcast of ds4 — no K=1
    outer-product matmuls.
"""

import numpy as np
from contextlib import ExitStack

import concourse.bacc as bacc
import concourse.mybir as mybir
import concourse.tile as tile
from concourse.bass import ds
from concourse.bass_utils import run_bass_kernel_spmd
from ml_dtypes import bfloat16

B, D, H = 65536, 64, 512
N_CORES = 8
B_CORE = B // N_CORES        # 8192 samples per core
CHUNK = 512                  # samples per pipeline chunk
N_CHUNKS = B_CORE // CHUNK   # 16
NT = H // 128                # 4 hidden-dim tiles of 128
ALPHA = 0.2

F32 = mybir.dt.float32
BF16 = mybir.dt.bfloat16
AF = mybir.ActivationFunctionType
OP = mybir.AluOpType

_PROGRAMS = {}


def _body(ctx, tc, uT_d, euT_d, eu4T_d, ezT_d, ezn_d, eu4_d, eun_d,
          bias_d, negb_d, negb4_d, out_d):
    nc = tc.nc
    wpool = ctx.enter_context(tc.tile_pool(name="weights", bufs=1))
    acts = ctx.enter_context(tc.tile_pool(name="acts", bufs=2))
    dspool = ctx.enter_context(tc.tile_pool(name="dsp", bufs=3))
    iop = ctx.enter_context(tc.tile_pool(name="io", bufs=2))
    utp = ctx.enter_context(tc.tile_pool(name="utp", bufs=3))
    pps = ctx.enter_context(tc.tile_pool(name="pps", bufs=4, space="PSUM"))
    pps4 = ctx.enter_context(tc.tile_pool(name="pps4", bufs=1, space="PSUM"))
    pdz = ctx.enter_context(tc.tile_pool(name="pdz", bufs=2, space="PSUM"))
    pgu = ctx.enter_context(tc.tile_pool(name="pgu", bufs=1, space="PSUM"))

    # ---- resident inputs (loaded once; uT streams per chunk) ----
    # Small tensors first so chunk-0 isn't gated behind the 6MB of wz
    # weights; wz loads are split per layer in first-use order.
    bias_s = wpool.tile([128, 4, NT], F32)
    nc.sync.dma_start(out=bias_s, in_=bias_d.rearrange("i (j p) -> p i j", p=128))
    negb_s = wpool.tile([128, 4, NT], F32)
    nc.sync.dma_start(out=negb_s, in_=negb_d.rearrange("i (j p) -> p i j", p=128))
    negb4_s = wpool.tile([1, 1], F32)
    nc.sync.dma_start(out=negb4_s, in_=negb4_d)
    euP_s = wpool.tile([128, 8 * 128], BF16)
    nc.sync.dma_start(out=euP_s, in_=euT_d)
    eu4T_s = wpool.tile([D, 1], BF16)
    nc.sync.dma_start(out=eu4T_s, in_=eu4T_d)
    ones_s = wpool.tile([128, 1], BF16)
    nc.vector.memset(ones_s, 1.0)
    eu4_s = wpool.tile([1, D], BF16)
    nc.sync.dma_start(out=eu4_s, in_=eu4_d)
    eun_s = wpool.tile([128, 4 * NT, D], BF16)
    nc.gpsimd.dma_start(out=eun_s, in_=eun_d.rearrange("b p d -> p b d"))
    zeros_s = wpool.tile([1, NT * D], BF16)
    nc.vector.memset(zeros_s, 0.0)
    ezT_v = ezT_d.rearrange("i (k p) n -> i p k n", p=128)
    ezT_s = wpool.tile([128, 3, NT, H], BF16)
    for i in range(3):
        nc.sync.dma_start(out=ezT_s[:, i], in_=ezT_v[i])
    ezn_v = ezn_d.rearrange("i (k p) n -> i p k n", p=128)
    ezn_s = wpool.tile([128, 3, NT, H], BF16)
    for i in (2, 1, 0):
        nc.gpsimd.dma_start(out=ezn_s[:, i], in_=ezn_v[i])

    out_v = out_d.rearrange("(c g p) d -> c p g d", g=NT, p=128)

    for c in range(N_CHUNKS):
        cs = ds(c * CHUNK, CHUNK)
        ut = utp.tile([128, CHUNK], BF16, name="ut")
        nc.gpsimd.dma_start(out=ut, in_=uT_d[:, cs])

        # ---------------- forward ----------------
        # u-path matmuls run as row-group pairs: lhsT halves live on SBUF
        # partitions 0-63 / 64-127 (euP), rhs is uT duplicated on both
        # halves, tile_position (0,0)/(64,0) -> the two K=64 matmuls
        # occupy disjoint quadrant rows and overlap on the PE array.
        # layer 0: z0 = lrelu(u @ E0.T + b0)^2; g0 = a0 * lrelu'(s0)
        z0 = acts.tile([128, NT, CHUNK], BF16, name="z0")
        g0 = acts.tile([128, NT, CHUNK], BF16, name="g0")
        for jp in range(NT // 2):
            pcols = ds((0 * 2 + jp) * 128, 128)
            sps = [pps.tile([128, CHUNK], F32, name="sp") for _ in range(2)]
            nc.tensor.matmul(sps[0], euP_s[0:64, pcols], ut[0:64, :],
                             tile_position=(0, 0), start=True, stop=True)
            nc.tensor.matmul(sps[1], euP_s[64:128, pcols], ut[64:128, :],
                             tile_position=(64, 0), start=True, stop=True)
            for h, sp in enumerate(sps):
                j = 2 * jp + h
                a0 = acts.tile([128, CHUNK], BF16, name="a0")
                nc.scalar.activation(a0, sp, AF.Prelu,
                                     bias=bias_s[:, 0, j:j + 1], alpha=ALPHA)
                nc.scalar.square(z0[:, j, :], a0)
                nc.scalar.activation(g0[:, j, :], a0, AF.Prelu, alpha=ALPHA)

        # layers 1..3: z_i = lrelu(u @ Eu_i.T + z_{i-1} @ Ez_i.T + b_i)
        zp = z0
        ms = {}
        for i in (1, 2, 3):
            zi = acts.tile([128, NT, CHUNK], BF16, name=f"z{i}")
            mi = acts.tile([128, NT, CHUNK], BF16, name=f"m{i}")
            for jp in range(NT // 2):
                pcols = ds((i * 2 + jp) * 128, 128)
                sps = [pps.tile([128, CHUNK], F32, name="sp") for _ in range(2)]
                nc.tensor.matmul(sps[0], euP_s[0:64, pcols], ut[0:64, :],
                                 tile_position=(0, 0), start=True, stop=False)
                nc.tensor.matmul(sps[1], euP_s[64:128, pcols], ut[64:128, :],
                                 tile_position=(64, 0), start=True, stop=False)
                for h, sp in enumerate(sps):
                    j = 2 * jp + h
                    for k in range(NT):
                        nc.tensor.matmul(sp, ezT_s[:, i - 1, k, ds(j * 128, 128)],
                                         zp[:, k, :], start=False,
                                         stop=(k == NT - 1))
                    nc.vector.tensor_scalar(mi[:, j, :], sp,
                                            negb_s[:, i, j:j + 1],
                                            ALPHA, OP.is_gt, OP.max)
                    nc.scalar.activation(zi[:, j, :], sp, AF.Prelu,
                                         bias=bias_s[:, i, j:j + 1], alpha=ALPHA)
            zp = zi
            ms[i] = mi

        # layer 4 (scalar head): only the lrelu' mask ds4 is needed
        s4p = pps4.tile([1, CHUNK], F32, name="s4p")
        nc.tensor.matmul(s4p, eu4T_s, ut[0:64, :], start=True, stop=False)
        for k in range(NT):
            nc.tensor.matmul(s4p, ones_s, zp[:, k, :],
                             start=False, stop=(k == NT - 1))
        ds4 = dspool.tile([1, CHUNK], BF16, name="ds4")
        nc.vector.tensor_scalar(ds4, s4p, negb4_s, ALPHA, OP.is_gt, OP.max)

        # ---------------- backward ----------------
        # grad accumulator in natural [samples, 64] layout; backward deltas
        # are the stationary operand so no output transpose is needed.
        gup = pgu.tile([128, NT, D], F32, name="gup")
        # single accumulation group over the whole bank: zero it with one
        # K=1 matmul (start=True), then everything accumulates into it.
        nc.tensor.matmul(gup[:, :, :], zeros_s[:, 0:128], zeros_s,
                         start=True, stop=False)
        for g in range(NT):
            nc.tensor.matmul(gup[:, g, :], ds4[:, ds(g * 128, 128)], eu4_s,
                             start=False, stop=False)

        # ds3 = broadcast(ds4) * m3   (Ez4 folded into layer-3 weights)
        bds4 = dspool.tile([128, CHUNK], BF16, name="bds4")
        nc.gpsimd.partition_broadcast(bds4, ds4)
        dst = {}
        for j in range(NT):
            dd = dspool.tile([128, CHUNK], BF16, name=f"ds3_{j}")
            nc.vector.tensor_tensor(dd, bds4, ms[3][:, j, :], OP.mult)
            dst[j] = dd

        for i in (3, 2, 1):
            # gu += ds_i @ Eu_i
            for j in range(NT):
                for g in range(NT):
                    nc.tensor.matmul(gup[:, g, :], dst[j][:, ds(g * 128, 128)],
                                     eun_s[:, i * NT + j, :],
                                     start=False, stop=False)
            # dz_{i-1} = ds_i @ Ez_i ; ds_{i-1} = dz * m_{i-1} (g0 for i==1)
            nxt = {}
            for j in range(NT):
                dzp = pdz.tile([128, CHUNK], F32, name="dzp")
                for k in range(NT):
                    nc.tensor.matmul(dzp, ezn_s[:, i - 1, k, ds(j * 128, 128)],
                                     dst[k], start=(k == 0), stop=(k == NT - 1))
                dd = dspool.tile([128, CHUNK], BF16, name=f"ds_{j}")
                mul = g0[:, j, :] if i == 1 else ms[i - 1][:, j, :]
                nc.vector.tensor_tensor(dd, dzp, mul, OP.mult)
                nxt[j] = dd
            dst = nxt

        # gu += ds0 @ (2*E0)  (factor 2 folded into eun block 0 on the host)
        for j in range(NT):
            for g in range(NT):
                nc.tensor.matmul(gup[:, g, :], dst[j][:, ds(g * 128, 128)],
                                 eun_s[:, j, :], start=False,
                                 stop=(j == NT - 1 and g == NT - 1))

        gsb = iop.tile([128, NT, D], F32, name="gsb")
        nc.scalar.copy(gsb, gup)
        nc.sync.dma_start(out=out_v[c], in_=gsb)


def _build_program():
    nc = bacc.Bacc("TRN2", target_bir_lowering=False, debug=False,
                   enable_asserts=False)
    uT_d = nc.dram_tensor("uT", [128, B_CORE], BF16, kind="ExternalInput").ap()
    euT_d = nc.dram_tensor("euT", [128, 8 * 128], BF16, kind="ExternalInput").ap()
    eu4T_d = nc.dram_tensor("eu4T", [D, 1], BF16, kind="ExternalInput").ap()
    ezT_d = nc.dram_tensor("ezT", [3, H, H], BF16, kind="ExternalInput").ap()
    ezn_d = nc.dram_tensor("ezn", [3, H, H], BF16, kind="ExternalInput").ap()
    eu4_d = nc.dram_tensor("eu4", [1, D], BF16, kind="ExternalInput").ap()
    eun_d = nc.dram_tensor("eun", [4 * NT, 128, D], BF16, kind="ExternalInput").ap()
    bias_d = nc.dram_tensor("bias", [4, H], F32, kind="ExternalInput").ap()
    negb_d = nc.dram_tensor("negb", [4, H], F32, kind="ExternalInput").ap()
    negb4_d = nc.dram_tensor("negb4", [1, 1], F32, kind="ExternalInput").ap()
    out_d = nc.dram_tensor("out", [B_CORE, D], F32, kind="ExternalOutput").ap()

    with ExitStack() as ctx:
        tc = ctx.enter_context(tile.TileContext(nc))
        _body(ctx, tc, uT_d, euT_d, eu4T_d, ezT_d, ezn_d, eu4_d, eun_d,
              bias_d, negb_d, negb4_d, out_d)
    nc.compile()
    return nc


def _get_program():
    if "main" not in _PROGRAMS:
        _PROGRAMS["main"] = _build_program()
    return _PROGRAMS["main"]


def _prepare_in_maps(inputs):
    u = np.asarray(inputs["u"], dtype=np.float32)
    wu = [np.asarray(inputs[f"wu{i}"], np.float32) for i in range(5)]
    wz = {i: np.asarray(inputs[f"wz{i}"], np.float32) for i in (1, 2, 3, 4)}
    b = [np.asarray(inputs[f"b{i}"], np.float32) for i in range(5)]

    Eu = [np.exp(w) for w in wu]           # [H, D]; Eu[4] is [1, D]
    Ez = {i: np.exp(wz[i]) for i in wz}    # [H, H]; Ez[4] is [1, H]

    # Fold Ez4 into layer 3 (the lrelu' mask is scale-invariant): layer-3
    # rows are scaled by Ez4, the L4 z-path weight becomes all-ones, and
    # backward's dz3 = broadcast(ds4).
    sc = Ez[4][0]                                                  # [H]
    Eu3s = Eu[3] * sc[:, None]
    Ez3s = Ez[3] * sc[:, None]
    b3s = b[3] * sc
    euT = np.concatenate(
        [Eu[0].T, Eu[1].T, Eu[2].T, Eu3s.T], axis=1)               # [D, 4H]
    # row-group pairs: pair p covers u-path tiles (2p, 2p+1) of the flat
    # (layer, j) order; halves live on partition rows 0-63 / 64-127.
    euP = np.empty((128, 8 * 128), np.float32)
    for p in range(8):
        euP[:D, p * 128:(p + 1) * 128] = euT[:, (2 * p) * 128:(2 * p + 1) * 128]
        euP[D:, p * 128:(p + 1) * 128] = euT[:, (2 * p + 1) * 128:(2 * p + 2) * 128]
    bias = np.stack([b[0], b[1], b[2], b3s])                       # [4, H]

    bf = lambda x: np.ascontiguousarray(x, dtype=np.float32).astype(bfloat16)
    f32 = lambda x: np.ascontiguousarray(x, dtype=np.float32)
    weights = {
        "euT": bf(euP),
        "eu4T": bf(Eu[4].T),
        "ezT": bf(np.stack([Ez[1].T, Ez[2].T, Ez3s.T])),
        "ezn": bf(np.stack([Ez[1], Ez[2], Ez3s])),
        "eu4": bf(Eu[4]),
        "eun": bf(np.concatenate([2.0 * Eu[0], Eu[1], Eu[2], Eu3s],
                                 axis=0).reshape(4 * NT, 128, D)),
        "bias": f32(bias),
        "negb": f32(-bias),
        "negb4": f32(-b[4].reshape(1, 1)),
    }

    in_maps = []
    for core in range(N_CORES):
        ush = u[core * B_CORE:(core + 1) * B_CORE]
        uT2 = np.concatenate([ush.T, ush.T], axis=0)               # [128, Bc]
        in_maps.append({"uT": bf(uT2), **weights})
    return in_maps


def kernel(**inputs):
    in_maps = _prepare_in_maps(inputs)
    nc = _get_program()
    res = run_bass_kernel_spmd(nc, in_maps, core_ids=list(range(N_CORES)))
    return np.concatenate([res.results[i]["out"] for i in range(N_CORES)],
                          axis=0)

